# revision 36
# baseline (speedup 1.0000x reference)
"""Multi-head attention with learned memory slots, 8-way sharded for TRN2.

Sharding: 8 cores = 4 batches x 2 (head-group, units-half) shards.
  core c -> batch b = c//2, parity g = c%2:
    - attention: computes heads 8g..8g+7 over all 1024 queries
    - output projection: computes ALL 16 heads x units columns
      512g..512(g+1), using Wo[:, half] from the host (column-split Wo
      => no reduce needed; cores exchange attention outputs instead)

All matmul operands are bf16 (host pre-casts inputs; fp32r runs at
2cy/col for contraction-64 / out-66 shapes on HW, bf16 is 1cy/col
everywhere + fast weight load). PSUM accumulation stays fp32.

Device kernel (identical SPMD program, per-core data differs):
  - x^T via HWDGE DMA-transpose (xbar), no PE transposes / evac copies
  - Q/K/V projections in bf16; K memory-slot columns via DMA-transpose
    of host-prescaled sqrt(m)*mk
  - per head: scores^T = K_h^T.T @ Q_h^T -> exp on ACT (bf16 out)
  - AV with ones-column appended to V gives softmax denominators in
    the same accumulation (out rows 0..63, sums row 64)
  - normalization: reciprocal_approx_fast of the sums row, ones-matmul
    partition broadcast, one elementwise multiply per head
  - head PAIRS packed on 128 partitions (odd head evacuated to
    partitions 64..127 via DVE cross-quadrant write) so the output
    projection contracts over 128 partitions
  - per pair: AllGather (pairwise) of the packed [128, 1024] bf16
    attention output DURING attention; Wo contracts all 16 heads from
    the gathered buffer; no tail collective
"""

import math
import os
from contextlib import ExitStack

import numpy as np

import concourse.bass as bass
import concourse.mybir as mybir
import concourse.tile as tile
from concourse import bacc
from concourse.bass_utils import run_bass_kernel_spmd

F32 = mybir.dt.float32
BF16 = mybir.dt.bfloat16
F32R = mybir.dt.float32r
NP_BF16 = mybir.dt.np(BF16)

B = 4
S = 1024          # sequence length (also #queries)
D = 1024          # model dim
NH = 8            # heads per core
DK = 64           # head dim
HD = NH * DK      # 512, per-core head*dim
M = 128           # memory slots
SKM = S + M       # 1152 keys incl. memory slots
NKC = SKM // 128  # 9 key chunks
UNITS = 1024
UH = UNITS // 2   # per-core output columns
SCALE_M = math.sqrt(float(M))
INV_SQRT_DK = 1.0 / math.sqrt(float(DK))

_CACHED = {}


def _f32r(ap):
    return ap.bitcast(F32R)


def _bcast_ap(ap, nparts):
    """Partition-broadcast AP: same free pattern on nparts partitions."""
    return bass.AP(tensor=ap.tensor, offset=ap.offset, ap=[[0, nparts]] + list(ap.ap))


def build_nc(debug=False):
    nc = bacc.Bacc("TRN2", target_bir_lowering=False, debug=False, num_devices=8)

    xq_e = nc.dram_tensor("xq", [S, D], BF16, kind="ExternalInput")
    xk_e = nc.dram_tensor("xk", [S, D], BF16, kind="ExternalInput")
    xv_e = nc.dram_tensor("xv", [S, D], BF16, kind="ExternalInput")
    wq_e = nc.dram_tensor("wq", [D, HD], BF16, kind="ExternalInput")
    wk_e = nc.dram_tensor("wk", [D, HD], BF16, kind="ExternalInput")
    wv_e = nc.dram_tensor("wv", [D, HD], BF16, kind="ExternalInput")
    bq_e = nc.dram_tensor("bq", [HD], F32, kind="ExternalInput")
    bk_e = nc.dram_tensor("bk", [HD], F32, kind="ExternalInput")
    bv_e = nc.dram_tensor("bv", [HD], F32, kind="ExternalInput")
    wo_e = nc.dram_tensor("wo", [2 * HD, UH], BF16, kind="ExternalInput")
    mk_e = nc.dram_tensor("mk", [M, HD], BF16, kind="ExternalInput")
    mv_e = nc.dram_tensor("mv", [M, HD], BF16, kind="ExternalInput")
    out_e = nc.dram_tensor("out", [S, UH], F32, kind="ExternalOutput")
    if debug:
        dbg_kt = nc.dram_tensor("dbg_kt", [128, 4, SKM], BF16,
                                kind="ExternalOutput")
        dbg_qt = nc.dram_tensor("dbg_qt", [128, 4, S], BF16, kind="ExternalOutput")
        dbg_es = nc.dram_tensor("dbg_es", [128, S], BF16, kind="ExternalOutput")
        dbg_sums = nc.dram_tensor("dbg_sums", [65, S], F32, kind="ExternalOutput")
        dbg_sumsbf = nc.dram_tensor("dbg_sumsbf", [65, S], BF16,
                                    kind="ExternalOutput")
        dbg_bc = nc.dram_tensor("dbg_bc", [128, S], F32, kind="ExternalOutput")
        dbg_pair = nc.dram_tensor("dbg_pair", [128, S], BF16,
                                  kind="ExternalOutput")
        dbg_gath = nc.dram_tensor("dbg_gath", [2, 128, S], BF16,
                                  kind="ExternalOutput")

    with tile.TileContext(nc) as tc, ExitStack() as ctx:
        consts = ctx.enter_context(tc.tile_pool(name="consts", bufs=1))
        dram = ctx.enter_context(tc.tile_pool(name="dram", bufs=1, space="DRAM"))

        # biases: bq/bk as [128, 4] per-partition scalars (hd on partitions)
        bq_t = consts.tile([128, 4], F32)
        bk_t = consts.tile([128, 4], F32)
        bv_bc = consts.tile([128, HD], F32)
        wo_sb = consts.tile([128, 8, UH], BF16)
        # bf16 ones row at partition 64: the K=1 lhsT of the recip-broadcast
        # matmuls (1.0 is exact in bf16)
        ones_t = consts.tile([65, 128], BF16)
        nc.vector.memset(ones_t, 1.0)

        # AllGather staging (per head: half a packed pair tile)
        stage_d = [dram.tile([64, S], BF16, name=f"stage{h}") for h in range(8)]
        gath_d = [dram.tile([2, 64, S], BF16, name=f"gath{h}") for h in range(8)]

        with tc.tile_pool(name="qkv", bufs=1) as qkv_pool, \
             tc.tile_pool(name="expS", bufs=18) as es_pool, \
             tc.tile_pool(name="score_ps", bufs=2, space="PSUM") as sc_pool:
            qT = qkv_pool.tile([128, 4, S], BF16)      # [hd_low, hd_grp, q]
            kT = qkv_pool.tile([128, 4, SKM], BF16)    # [hd_low, hd_grp, k]
            vt = qkv_pool.tile([128, NKC, NH * 66], BF16)  # [k_low, kc, h*66]
            # gathered attention outputs: [part, (G, pair), q]
            outT_all = qkv_pool.tile([128, 8, S], BF16)

            # V layout: head block h = 66 cols: [V_h(64) | ones | ones]
            vt_r = vt[:].rearrange("p kc (h c) -> p kc h c", c=66)
            nc.vector.memset(vt_r[:, :, :, 64:66], 1.0)

            # ---- input transposes (HWDGE xbar DMA) -----------------------
            # one whole-input DMA: out[p, dc, s] = in[s, dc*128+p]; the
            # contiguous 2KB source rows keep the xbar near full rate
            def transpose_in(x_ext, xT, eng):
                eng.dma_start(out=xT, in_=x_ext[:], transpose=True)

            es_tiles = {}

            def emit_scores(h):
                hw, hp = h // 2, 64 * (h % 2)
                for kc in range(NKC):
                    sc_ps = sc_pool.tile([128, S], F32, tag="sc")
                    lhsT = kT[hp:hp + 64, hw, kc * 128:(kc + 1) * 128]
                    for nq in range(2):
                        nc.tensor.matmul(
                            sc_ps[:, nq * 512:(nq + 1) * 512],
                            lhsT,
                            qT[hp:hp + 64, hw, nq * 512:(nq + 1) * 512],
                            start=True, stop=True,
                        )
                    es = es_pool.tile([128, S], BF16, tag="es")
                    nc.scalar.activation(
                        es, sc_ps, mybir.ActivationFunctionType.Exp,
                        scale=INV_SQRT_DK,
                    )
                    if debug and h == 0 and kc == 0:
                        nc.sync.dma_start(out=dbg_es[:], in_=es)
                    es_tiles[(h, kc)] = es

            def emit_av(h, outp):
                for kc in range(NKC):
                    vh = vt[:, kc, 66 * h:66 * h + 66]
                    es = es_tiles.pop((h, kc))
                    for nq in range(2):
                        nc.tensor.matmul(
                            outp[0:66, nq * 512:(nq + 1) * 512],
                            vh,
                            es[:, nq * 512:(nq + 1) * 512],
                            start=(kc == 0), stop=(kc == NKC - 1),
                        )

            def emit_scores_av(h, outp):
                """Steady state (h>=2): per kc, scores -> exp -> AV."""
                hw, hp = h // 2, 64 * (h % 2)
                for kc in range(NKC):
                    sc_ps = sc_pool.tile([128, S], F32, tag="sc")
                    lhsT = kT[hp:hp + 64, hw, kc * 128:(kc + 1) * 128]
                    for nq in range(2):
                        nc.tensor.matmul(
                            sc_ps[:, nq * 512:(nq + 1) * 512],
                            lhsT,
                            qT[hp:hp + 64, hw, nq * 512:(nq + 1) * 512],
                            start=True, stop=True,
                        )
                    es = es_pool.tile([128, S], BF16, tag="es")
                    nc.scalar.activation(
                        es, sc_ps, mybir.ActivationFunctionType.Exp,
                        scale=INV_SQRT_DK,
                    )
                    vh = vt[:, kc, 66 * h:66 * h + 66]
                    for nq in range(2):
                        nc.tensor.matmul(
                            outp[0:66, nq * 512:(nq + 1) * 512],
                            vh,
                            es[:, nq * 512:(nq + 1) * 512],
                            start=(kc == 0), stop=(kc == NKC - 1),
                        )

            # ---- projections ---------------------------------------------
            with tc.tile_pool(name="wproj", bufs=3) as wpool, \
                 tc.tile_pool(name="xT", bufs=3) as xT_pool, \
                 tc.tile_pool(name="proj_ps", bufs=2, space="PSUM") as proj_pool:

                xqT = xT_pool.tile([128, 8, S], BF16, tag="xT")
                xkT = xT_pool.tile([128, 8, S], BF16, tag="xT")
                xvT = xT_pool.tile([128, 8, S], BF16, tag="xT")
                w_tiles = {}
                for name in ("wq", "wk", "wv"):
                    w_tiles[name] = wpool.tile([128, 8, HD], BF16, tag="w",
                                               name=f"w_{name}")

                # Startup-ordered DMA emission.  The xbar transpose waits for
                # every outstanding prior DMA transfer, so the transposes go
                # FIRST; weights ride the SWDGE queue in parallel; small
                # consts last.
                # K memory-slot columns: host-prescaled sqrt(m)*mk, transposed
                nc.sync.dma_start(out=kT[:, :, S:SKM], in_=mk_e[:],
                                  transpose=True)
                transpose_in(xq_e, xqT, nc.sync)
                transpose_in(xk_e, xkT, nc.scalar)
                transpose_in(xv_e, xvT, nc.sync)
                for name, w_ext in (("wq", wq_e), ("wk", wk_e), ("wv", wv_e)):
                    nc.gpsimd.dma_start(
                        out=w_tiles[name],
                        in_=w_ext[:].rearrange("(dc p) c -> p dc c", p=128))
                nc.gpsimd.dma_start(
                    out=bq_t, in_=bq_e[:].rearrange("(mt p) -> p mt", p=128))
                nc.gpsimd.dma_start(
                    out=bk_t, in_=bk_e[:].rearrange("(mt p) -> p mt", p=128))
                nc.gpsimd.dma_start(out=bv_bc, in_=_bcast_ap(bv_e[:], 128))
                # memory-slot rows of V: host-prescaled sqrt(m)*mv
                nc.gpsimd.dma_start(
                    out=vt_r[:, NKC - 1, :, 0:64],
                    in_=mv_e[:].rearrange("p (h c) -> p h c", c=64),
                )
                # Wo packed by head pairs: dram row (G*8 + 2*hp + t)*64 + d ->
                # partition t*64+d, free (G*4+hp, c).  G = group, hp = pair.
                nc.gpsimd.dma_start(
                    out=wo_sb,
                    in_=wo_e[:].rearrange("(G hp t d) c -> (t d) (G hp) c",
                                          G=2, hp=4, t=2),
                )

                # Q then K: two mt per group, dc-accumulated
                for name, xT, dstT, bias in (("wq", xqT, qT, bq_t),
                                             ("wk", xkT, kT, bk_t)):
                    w_t = w_tiles[name]
                    for grp in range(2):
                        ps = [proj_pool.tile([128, S], F32, tag="proj",
                                             name=f"ps{name}{grp}{mi}")
                              for mi in range(2)]
                        for dc in range(8):
                            for mi in range(2):
                                mt = grp * 2 + mi
                                lhsT = w_t[:, dc, mt * 128:(mt + 1) * 128]
                                for nq in range(2):
                                    nc.tensor.matmul(
                                        ps[mi][:, nq * 512:(nq + 1) * 512],
                                        lhsT,
                                        xT[:, dc, nq * 512:(nq + 1) * 512],
                                        start=(dc == 0), stop=(dc == 7),
                                    )
                        for mi in range(2):
                            mt = grp * 2 + mi
                            nc.vector.tensor_scalar_add(
                                dstT[:, mt, 0:S], ps[mi], bias[:, mt:mt + 1]
                            )
                    if name == "wk":
                        if debug:
                            nc.sync.dma_start(out=dbg_qt[:], in_=qT)
                            nc.sync.dma_start(out=dbg_kt[:], in_=kT)
                        # qT/kT heads 0,1 ready after group 0 of K: emit the
                        # first two heads' scores+exp so ACT fills while PE
                        # finishes K group1 + V projection
                        emit_scores(0)
                        emit_scores(1)

                # V: [seq, hd] packed into 66-col head blocks
                w_t = w_tiles["wv"]
                for st in range(8):
                    ps = proj_pool.tile([128, HD], F32, tag="proj")
                    for dc in range(8):
                        nc.tensor.matmul(
                            ps,
                            xvT[:, dc, st * 128:(st + 1) * 128],
                            w_t[:, dc, :],
                            start=(dc == 0), stop=(dc == 7),
                        )
                    nc.vector.tensor_add(
                        vt_r[:, st, :, 0:64],
                        ps[:].rearrange("p (h c) -> p h c", c=64),
                        bv_bc[:].rearrange("p (h c) -> p h c", c=64),
                    )

            # ---- attention -----------------------------------------------
            with tc.tile_pool(name="av_ps", bufs=2, space="PSUM") as av_pool, \
                 tc.tile_pool(name="sums", bufs=2) as sums_pool, \
                 tc.tile_pool(name="bc", bufs=2) as bc_pool, \
                 tc.tile_pool(name="pairT", bufs=2) as pair_pool:
                pair_tiles = {}
                sums_tiles = {}
                bc_tiles = {}

                def normalize(hh):
                    """Broadcast raw sums(hh) to 128 partitions, reciprocal
                    on the full-partition tile (the 1-partition custom-DVE
                    op writes nothing on HW), then scale."""
                    sums_t = sums_tiles.pop(hh)
                    bc_ps = av_pool.tile([128, S], F32, tag="av")
                    for nq in range(2):
                        nc.tensor.matmul(
                            bc_ps[:, nq * 512:(nq + 1) * 512],
                            ones_t[64:65, 0:128],
                            sums_t[64:65, nq * 512:(nq + 1) * 512],
                            start=True, stop=True,
                        )
                    bc_sb = bc_pool.tile([128, S], F32, tag="bc")
                    nc.vector.reciprocal_approx_fast(out=bc_sb, in_=bc_ps)
                    po = 64 * (hh % 2)
                    pt = pair_tiles[hh // 2]
                    if debug and hh == 0:
                        nc.sync.dma_start(out=dbg_bc[:], in_=bc_sb)
                    nc.vector.tensor_mul(
                        pt[po:po + 64, :], pt[po:po + 64, :], bc_sb[po:po + 64, :]
                    )

                def stage_head(hh):
                    """Exchange one normalized head (half a pair tile)."""
                    po = 64 * (hh % 2)
                    if hh % 2 == 0:
                        pt = pair_tiles[hh // 2]
                    else:
                        pt = pair_tiles.pop(hh // 2)
                    if debug and hh == 1:
                        nc.sync.dma_start(out=dbg_pair[:], in_=pt)
                    nc.sync.dma_start(out=stage_d[hh][:], in_=pt[po:po + 64, :])
                    nc.gpsimd.collective_compute(
                        "AllGather",
                        mybir.AluOpType.bypass,
                        replica_groups=[[0, 1], [2, 3], [4, 5], [6, 7]],
                        ins=[stage_d[hh][:].opt()],
                        outs=[gath_d[hh][:].opt()],
                    )
                    for gi in range(2):
                        nc.sync.dma_start(
                            out=outT_all[po:po + 64, gi * 4 + hh // 2, :],
                            in_=gath_d[hh][gi, :, :],
                        )

                for h in range(NH):
                    outp = av_pool.tile([128, S], F32, tag="av")
                    if h < 2:
                        emit_av(h, outp)
                    else:
                        emit_scores_av(h, outp)
                    # evacuate: even head -> partitions 0..63, odd -> 64..127
                    if h % 2 == 0:
                        pt = pair_pool.tile([128, S], BF16, tag="pair")
                        pair_tiles[h // 2] = pt
                    else:
                        pt = pair_tiles[h // 2]
                    po = 64 * (h % 2)
                    nc.vector.tensor_copy(pt[po:po + 64, :], outp[0:64, :])
                    # raw sums row off PSUM as bf16 (broadcast matmul operand)
                    sums_t = sums_pool.tile([65, S], BF16, tag="sums")
                    sums_tiles[h] = sums_t
                    nc.vector.tensor_copy(sums_t[64:65, :], outp[64:65, :])
                    if debug and h == 0:
                        nc.sync.dma_start(out=dbg_sumsbf[:], in_=sums_t)
                    if h > 0:
                        normalize(h - 1)
                        stage_head(h - 1)
                normalize(NH - 1)
                stage_head(NH - 1)

        # ---- output projection (contraction 128 over head pairs) ---------
        with tc.tile_pool(name="wo_ps", bufs=8, space="PSUM") as wo_ps_pool, \
             tc.tile_pool(name="osb", bufs=2) as o_pool:
            wops = [wo_ps_pool.tile([128, UH], F32, tag="wops",
                                    name=f"wops{mt}")
                    for mt in range(8)]
            # pairs 0..2 accumulate as their gathers land; the last pair is
            # applied mt-by-mt so evac/DMA of mt k overlaps matmuls of k+1
            for i, (p, gi) in enumerate(
                    [(p, gi) for p in range(3) for gi in range(2)]):
                for mt in range(8):
                    nc.tensor.matmul(
                        wops[mt],
                        outT_all[:, gi * 4 + p, mt * 128:(mt + 1) * 128],
                        wo_sb[:, gi * 4 + p, :],
                        start=(i == 0), stop=False,
                    )
            for mt in range(8):
                for gi in range(2):
                    nc.tensor.matmul(
                        wops[mt],
                        outT_all[:, gi * 4 + 3, mt * 128:(mt + 1) * 128],
                        wo_sb[:, gi * 4 + 3, :],
                        start=False, stop=(gi == 1),
                    )
                # bo is added on the host; ACT is idle here so it evacuates
                osb = o_pool.tile([128, UH], F32, tag="osb")
                nc.scalar.copy(osb, wops[mt])
                nc.sync.dma_start(
                    out=out_e[mt * 128:(mt + 1) * 128, :], in_=osb
                )

    nc.compile()
    return nc


def _get_nc():
    if "nc" not in _CACHED:
        _CACHED["nc"] = build_nc()
    return _CACHED["nc"]


def _in_maps(queries, keys, values, Wq, bq, Wk, bk, Wv, bv, Wo, bo, mk, mv):
    x_bf = [np.ascontiguousarray(a).astype(NP_BF16)
            for a in (queries, keys, values)]
    w_bf = [np.ascontiguousarray(a).astype(NP_BF16) for a in (Wq, Wk, Wv)]
    wo_bf = np.ascontiguousarray(Wo).astype(NP_BF16)
    mk_bf = np.ascontiguousarray(SCALE_M * mk).astype(NP_BF16)
    mv_bf = np.ascontiguousarray(SCALE_M * mv).astype(NP_BF16)
    maps = []
    for c in range(8):
        b, g = c // 2, c % 2
        sl = slice(g * HD, (g + 1) * HD)
        ul = slice(g * UH, (g + 1) * UH)
        maps.append({
            "xq": x_bf[0][b],
            "xk": x_bf[1][b],
            "xv": x_bf[2][b],
            "wq": np.ascontiguousarray(w_bf[0][:, sl]),
            "wk": np.ascontiguousarray(w_bf[1][:, sl]),
            "wv": np.ascontiguousarray(w_bf[2][:, sl]),
            "bq": np.ascontiguousarray(bq[sl]),
            "bk": np.ascontiguousarray(bk[sl]),
            "bv": np.ascontiguousarray(bv[sl]),
            "wo": np.ascontiguousarray(wo_bf[:, ul]),
            "mk": np.ascontiguousarray(mk_bf[:, sl]),
            "mv": np.ascontiguousarray(mv_bf[:, sl]),
        })
    return maps


def kernel(queries, keys, values, Wq, bq, Wk, bk, Wv, bv, Wo, bo, mk, mv, h=16,
           **_unused):
    queries = np.asarray(queries, np.float32)
    keys = np.asarray(keys, np.float32)
    values = np.asarray(values, np.float32)
    Wq = np.asarray(Wq, np.float32)
    Wk = np.asarray(Wk, np.float32)
    Wv = np.asarray(Wv, np.float32)
    Wo = np.asarray(Wo, np.float32)
    bq = np.asarray(bq, np.float32)
    bk = np.asarray(bk, np.float32)
    bv = np.asarray(bv, np.float32)
    bo = np.asarray(bo, np.float32)
    mk = np.asarray(mk, np.float32).reshape(M, -1)
    mv = np.asarray(mv, np.float32).reshape(M, -1)

    nc = _get_nc()
    in_maps = _in_maps(queries, keys, values, Wq, bq, Wk, bk, Wv, bv, Wo, bo,
                       mk, mv)

    trace = bool(int(os.environ.get("BASS_KERNEL_TRACE", "0")))
    res = run_bass_kernel_spmd(nc, in_maps, list(range(8)), trace=trace)
    _CACHED["last_result"] = res

    out = np.empty((B, S, UNITS), np.float32)
    for c in range(8):
        b, g = c // 2, c % 2
        out[b, :, g * UH:(g + 1) * UH] = res.results[c]["out"] + bo[g * UH:(g + 1) * UH]
    return out


# revision 73
# speedup vs baseline: 1.2872x; 1.2872x over previous
"""Multi-head attention with learned memory slots, 8-way sharded for TRN2.

Sharding: 8 cores = 4 batches x 2 (head-group, units-half) shards.
  core c -> batch b = c//2, parity g = c%2:
    - attention: computes heads 8g..8g+7 over all 1024 queries
    - output projection: computes ALL 16 heads x units columns
      512g..512(g+1), using Wo[:, half] from the host (column-split Wo
      => no reduce needed; cores exchange attention outputs instead)

All matmul operands are bf16 (host pre-casts inputs; fp32r runs at
2cy/col for contraction-64 / out-66 shapes on HW, bf16 is 1cy/col
everywhere + fast weight load). PSUM accumulation stays fp32.

Device kernel (identical SPMD program, per-core data differs):
  - x^T via HWDGE DMA-transpose (xbar), no PE transposes / evac copies
  - Q/K/V projections in bf16; K memory-slot columns via DMA-transpose
    of host-prescaled sqrt(m)*mk
  - per head: scores^T = K_h^T.T @ Q_h^T -> exp on ACT (bf16 out)
  - AV with ones-column appended to V gives softmax denominators in
    the same accumulation (out rows 0..63, sums row 64)
  - normalization: reciprocal_approx_fast of the sums row, ones-matmul
    partition broadcast, one elementwise multiply per head
  - head PAIRS packed on 128 partitions (odd head evacuated to
    partitions 64..127 via DVE cross-quadrant write) so the output
    projection contracts over 128 partitions
  - per pair: AllGather (pairwise) of the packed [128, 1024] bf16
    attention output DURING attention; Wo contracts all 16 heads from
    the gathered buffer; no tail collective
"""

import math
import os
from contextlib import ExitStack

import numpy as np

import concourse.bass as bass
import concourse.mybir as mybir
import concourse.tile as tile
from concourse import bacc
from concourse.bass_utils import run_bass_kernel_spmd

F32 = mybir.dt.float32
BF16 = mybir.dt.bfloat16
F32R = mybir.dt.float32r
NP_BF16 = mybir.dt.np(BF16)

B = 4
S = 1024          # sequence length (also #queries)
D = 1024          # model dim
NH = 8            # heads per core
DK = 64           # head dim
HD = NH * DK      # 512, per-core head*dim
M = 128           # memory slots
SKM = S + M       # 1152 keys incl. memory slots
NKC = SKM // 128  # 9 key chunks
UNITS = 1024
UH = UNITS // 2   # per-core output columns
SCALE_M = math.sqrt(float(M))
INV_SQRT_DK = 1.0 / math.sqrt(float(DK))

_CACHED = {}


def _f32r(ap):
    return ap.bitcast(F32R)


def _bcast_ap(ap, nparts):
    """Partition-broadcast AP: same free pattern on nparts partitions."""
    return bass.AP(tensor=ap.tensor, offset=ap.offset, ap=[[0, nparts]] + list(ap.ap))


def build_nc(debug=False):
    nc = bacc.Bacc("TRN2", target_bir_lowering=False, debug=False, num_devices=8)

    # Everything arrives PRE-SWIZZLED from the host in the exact SBUF tile
    # layout, so every load is a flat contiguous DMA (partition-strided
    # sources measured only ~85 GB/s; contiguous runs near HBM rate).
    xq_e = nc.dram_tensor("xq", [128, 8, S], BF16, kind="ExternalInput")
    xk_e = nc.dram_tensor("xk", [128, 8, S], BF16, kind="ExternalInput")
    xv_e = nc.dram_tensor("xv", [128, 8, S], BF16, kind="ExternalInput")
    wq_e = nc.dram_tensor("wq", [128, 8, HD], BF16, kind="ExternalInput")
    wk_e = nc.dram_tensor("wk", [128, 8, HD], BF16, kind="ExternalInput")
    wv_e = nc.dram_tensor("wv", [128, 8, HD], BF16, kind="ExternalInput")
    bq_e = nc.dram_tensor("bq", [128, 4], F32, kind="ExternalInput")
    bk_e = nc.dram_tensor("bk", [128, 4], F32, kind="ExternalInput")
    bv_e = nc.dram_tensor("bv", [HD], F32, kind="ExternalInput")
    wo_e = nc.dram_tensor("wo", [128, 8, UH], BF16, kind="ExternalInput")
    mk_e = nc.dram_tensor("mk", [128, 4, M], BF16, kind="ExternalInput")
    mv_e = nc.dram_tensor("mv", [M, HD], BF16, kind="ExternalInput")
    out_e = nc.dram_tensor("out", [S, UH], F32, kind="ExternalOutput")
    if debug:
        dbg_kt = nc.dram_tensor("dbg_kt", [128, 4, SKM], BF16,
                                kind="ExternalOutput")
        dbg_qt = nc.dram_tensor("dbg_qt", [128, 4, S], BF16, kind="ExternalOutput")
        dbg_es = nc.dram_tensor("dbg_es", [128, S], BF16, kind="ExternalOutput")
        dbg_sums = nc.dram_tensor("dbg_sums", [65, S], F32, kind="ExternalOutput")
        dbg_sumsbf = nc.dram_tensor("dbg_sumsbf", [65, S], BF16,
                                    kind="ExternalOutput")
        dbg_bc = nc.dram_tensor("dbg_bc", [128, S], F32, kind="ExternalOutput")
        dbg_pair = nc.dram_tensor("dbg_pair", [128, S], BF16,
                                  kind="ExternalOutput")
        dbg_gath = nc.dram_tensor("dbg_gath", [2, 128, S], BF16,
                                  kind="ExternalOutput")
        dbg_outall = nc.dram_tensor("dbg_outall", [128, 8, S], BF16,
                                    kind="ExternalOutput")

    with tile.TileContext(nc) as tc, ExitStack() as ctx:
        consts = ctx.enter_context(tc.tile_pool(name="consts", bufs=1))
        dram = ctx.enter_context(tc.tile_pool(name="dram", bufs=1, space="DRAM"))

        # biases: bq/bk as [128, 4] per-partition scalars (hd on partitions)
        bq_t = consts.tile([128, 4], F32)
        bk_t = consts.tile([128, 4], F32)
        bv_bc = consts.tile([128, HD], F32)
        wo_sb = consts.tile([128, 8, UH], BF16)
        # bf16 ones row at partition 64: the K=1 lhsT of the recip-broadcast
        # matmuls (1.0 is exact in bf16)
        ones_t = consts.tile([65, 128], BF16)
        nc.vector.memset(ones_t, 1.0)

        # AllGather staging: pairs 0..2 leave as soon as each pair is
        # normalized (hidden under attention); heads 6 and 7 leave
        # separately so the tail exchange is half-size
        stage_ps = [dram.tile([128, S], BF16, name=f"stagep{p}")
                    for p in range(3)]
        gath_ps = [dram.tile([2, 128, S], BF16, name=f"gathp{p}")
                   for p in range(3)]
        stage_hs = [dram.tile([64, S], BF16, name=f"stageh{t}")
                    for t in range(2)]
        gath_hs = [dram.tile([2, 64, S], BF16, name=f"gathh{t}")
                   for t in range(2)]

        with tc.tile_pool(name="qkv", bufs=1) as qkv_pool, \
             tc.tile_pool(name="expS", bufs=24) as es_pool, \
             tc.tile_pool(name="score_ps", bufs=2, space="PSUM") as sc_pool:
            qT = qkv_pool.tile([128, 4, S], BF16)      # [hd_low, hd_grp, q]
            kT = qkv_pool.tile([128, 4, SKM], BF16)    # [hd_low, hd_grp, k]
            vt = qkv_pool.tile([128, NKC, NH * 66], BF16)  # [k_low, kc, h*66]
            # gathered attention outputs: [part, (G, pair), q]
            outT_all = qkv_pool.tile([128, 8, S], BF16)

            # V layout: head block h = 66 cols: [V_h(64) | ones | ones]
            vt_r = vt[:].rearrange("p kc (h c) -> p kc h c", c=66)
            nc.vector.memset(vt_r[:, :, :, 64:66], 1.0)

            # x^T comes pre-swizzled from the host: flat full-rate DMA
            def transpose_in(x_ext, xT, eng):
                eng.dma_start(out=xT, in_=x_ext[:])

            es_tiles = {}

            def emit_scores(h):
                hw, hp = h // 2, 64 * (h % 2)
                for kc in range(NKC):
                    sc_ps = sc_pool.tile([128, S], F32, tag="sc")
                    lhsT = kT[hp:hp + 64, hw, kc * 128:(kc + 1) * 128]
                    for nq in range(2):
                        nc.tensor.matmul(
                            sc_ps[:, nq * 512:(nq + 1) * 512],
                            lhsT,
                            qT[hp:hp + 64, hw, nq * 512:(nq + 1) * 512],
                            start=True, stop=True,
                        )
                    es = es_pool.tile([128, S], BF16, tag="es")
                    nc.scalar.activation(
                        es, sc_ps, mybir.ActivationFunctionType.Exp,
                        scale=INV_SQRT_DK,
                    )
                    if debug and h == 0 and kc == 0:
                        nc.sync.dma_start(out=dbg_es[:], in_=es)
                    es_tiles[(h, kc)] = es

            def emit_av(h, outp):
                for kc in range(NKC):
                    vh = vt[:, kc, 66 * h:66 * h + 66]
                    es = es_tiles.pop((h, kc))
                    for nq in range(2):
                        nc.tensor.matmul(
                            outp[0:66, nq * 512:(nq + 1) * 512],
                            vh,
                            es[:, nq * 512:(nq + 1) * 512],
                            start=(kc == 0), stop=(kc == NKC - 1),
                        )

            def emit_scores_av(h, outp):
                """Steady state (h>=2): per kc, scores -> exp -> AV."""
                hw, hp = h // 2, 64 * (h % 2)
                for kc in range(NKC):
                    sc_ps = sc_pool.tile([128, S], F32, tag="sc")
                    lhsT = kT[hp:hp + 64, hw, kc * 128:(kc + 1) * 128]
                    for nq in range(2):
                        nc.tensor.matmul(
                            sc_ps[:, nq * 512:(nq + 1) * 512],
                            lhsT,
                            qT[hp:hp + 64, hw, nq * 512:(nq + 1) * 512],
                            start=True, stop=True,
                        )
                    es = es_pool.tile([128, S], BF16, tag="es")
                    nc.scalar.activation(
                        es, sc_ps, mybir.ActivationFunctionType.Exp,
                        scale=INV_SQRT_DK,
                    )
                    vh = vt[:, kc, 66 * h:66 * h + 66]
                    for nq in range(2):
                        nc.tensor.matmul(
                            outp[0:66, nq * 512:(nq + 1) * 512],
                            vh,
                            es[:, nq * 512:(nq + 1) * 512],
                            start=(kc == 0), stop=(kc == NKC - 1),
                        )

            # ---- projections ---------------------------------------------
            with tc.tile_pool(name="wproj", bufs=3) as wpool, \
                 tc.tile_pool(name="xT", bufs=3) as xT_pool, \
                 tc.tile_pool(name="proj_ps", bufs=2, space="PSUM") as proj_pool:

                xqT = xT_pool.tile([128, 8, S], BF16, tag="xT")
                xkT = xT_pool.tile([128, 8, S], BF16, tag="xT")
                xvT = xT_pool.tile([128, 8, S], BF16, tag="xT")
                w_tiles = {}
                for name in ("wq", "wk", "wv"):
                    w_tiles[name] = wpool.tile([128, 8, HD], BF16, tag="w",
                                               name=f"w_{name}")

                # DMA emission: x^T chain on sync (xq in halves so Q proj
                # can start after the first MB), weights chain on scalar.
                # Within a queue, ring backpressure keeps transfers roughly
                # in priority order.
                nc.sync.dma_start(out=xqT[:, 0:4, :], in_=xq_e[:, 0:4, :])
                nc.sync.dma_start(out=xqT[:, 4:8, :], in_=xq_e[:, 4:8, :])
                transpose_in(xk_e, xkT, nc.sync)
                transpose_in(xv_e, xvT, nc.sync)
                nc.scalar.dma_start(out=w_tiles["wq"], in_=wq_e[:])
                nc.scalar.dma_start(out=w_tiles["wk"], in_=wk_e[:])
                nc.scalar.dma_start(out=w_tiles["wv"], in_=wv_e[:])
                # K memory-slot columns: host-preswizzled sqrt(m)*mk^T
                nc.gpsimd.dma_start(out=kT[:, :, S:SKM], in_=mk_e[:])
                nc.gpsimd.dma_start(out=bq_t, in_=bq_e[:])
                nc.gpsimd.dma_start(out=bk_t, in_=bk_e[:])
                nc.gpsimd.dma_start(out=bv_bc, in_=_bcast_ap(bv_e[:], 128))
                # memory-slot rows of V: host-prescaled sqrt(m)*mv
                nc.gpsimd.dma_start(
                    out=vt_r[:, NKC - 1, :, 0:64],
                    in_=mv_e[:].rearrange("p (h c) -> p h c", c=64),
                )
                # Wo pre-packed by head pairs on the host
                nc.gpsimd.dma_start(out=wo_sb, in_=wo_e[:])

                # Q then K: two mt per group, dc-accumulated.  Scores for
                # heads 0..3 are emitted as soon as their kT group lands so
                # ACT streams exps while PE runs the rest of the projections.
                for name, xT, dstT, bias in (("wq", xqT, qT, bq_t),
                                             ("wk", xkT, kT, bk_t)):
                    w_t = w_tiles[name]
                    for grp in range(2):
                        ps = [proj_pool.tile([128, S], F32, tag="proj",
                                             name=f"ps{name}{grp}{mi}")
                              for mi in range(2)]
                        for dc in range(8):
                            for mi in range(2):
                                mt = grp * 2 + mi
                                lhsT = w_t[:, dc, mt * 128:(mt + 1) * 128]
                                for nq in range(2):
                                    nc.tensor.matmul(
                                        ps[mi][:, nq * 512:(nq + 1) * 512],
                                        lhsT,
                                        xT[:, dc, nq * 512:(nq + 1) * 512],
                                        start=(dc == 0), stop=(dc == 7),
                                    )
                        for mi in range(2):
                            mt = grp * 2 + mi
                            nc.vector.tensor_scalar_add(
                                dstT[:, mt, 0:S], ps[mi], bias[:, mt:mt + 1]
                            )
                        if name == "wk":
                            if debug and grp == 1:
                                nc.sync.dma_start(out=dbg_qt[:], in_=qT)
                                nc.sync.dma_start(out=dbg_kt[:], in_=kT)
                            emit_scores(grp * 2)
                            emit_scores(grp * 2 + 1)

                # V: [seq, hd] packed into 66-col head blocks
                w_t = w_tiles["wv"]
                for st in range(8):
                    ps = proj_pool.tile([128, HD], F32, tag="proj")
                    for dc in range(8):
                        nc.tensor.matmul(
                            ps,
                            xvT[:, dc, st * 128:(st + 1) * 128],
                            w_t[:, dc, :],
                            start=(dc == 0), stop=(dc == 7),
                        )
                    nc.vector.tensor_add(
                        vt_r[:, st, :, 0:64],
                        ps[:].rearrange("p (h c) -> p h c", c=64),
                        bv_bc[:].rearrange("p (h c) -> p h c", c=64),
                    )

            # ---- attention -----------------------------------------------
            with tc.tile_pool(name="av_ps", bufs=2, space="PSUM") as av_pool, \
                 tc.tile_pool(name="sums", bufs=2) as sums_pool, \
                 tc.tile_pool(name="bc", bufs=2) as bc_pool, \
                 tc.tile_pool(name="pairT", bufs=3) as pair_pool:
                pair_tiles = {}
                sums_tiles = {}
                bc_tiles = {}

                def normalize(hh):
                    """Broadcast raw sums(hh) to 128 partitions, reciprocal
                    on the full-partition tile (the 1-partition custom-DVE
                    op writes nothing on HW), then scale."""
                    sums_t = sums_tiles.pop(hh)
                    bc_ps = av_pool.tile([128, S], F32, tag="av")
                    for nq in range(2):
                        nc.tensor.matmul(
                            bc_ps[:, nq * 512:(nq + 1) * 512],
                            ones_t[64:65, 0:128],
                            sums_t[64:65, nq * 512:(nq + 1) * 512],
                            start=True, stop=True,
                        )
                    bc_sb = bc_pool.tile([128, S], F32, tag="bc")
                    nc.vector.reciprocal_approx_fast(out=bc_sb, in_=bc_ps)
                    po = 64 * (hh % 2)
                    pt = pair_tiles[hh // 2]
                    if debug and hh == 0:
                        nc.sync.dma_start(out=dbg_bc[:], in_=bc_sb)
                    nc.vector.tensor_mul(
                        pt[po:po + 64, :], pt[po:po + 64, :], bc_sb[po:po + 64, :]
                    )

                def ag(stage_t, gath_t):
                    nc.gpsimd.collective_compute(
                        "AllGather",
                        mybir.AluOpType.bypass,
                        replica_groups=[[0, 1], [2, 3], [4, 5], [6, 7]],
                        ins=[stage_t[:].opt()],
                        outs=[gath_t[:].opt()],
                    )

                # gather loads ride the SWDGE queue: a load must wait for
                # its collective, and on the in-order sync queue that wait
                # would block every later stage DMA
                def stage_pair(p):
                    pt = pair_tiles.pop(p)
                    if debug and p == 0:
                        nc.sync.dma_start(out=dbg_pair[:], in_=pt)
                    nc.sync.dma_start(out=stage_ps[p][:], in_=pt)
                    ag(stage_ps[p], gath_ps[p])
                    for gi in range(2):
                        nc.gpsimd.dma_start(
                            out=outT_all[:, gi * 4 + p, :],
                            in_=gath_ps[p][gi, :, :],
                        )

                def stage_head67(t):
                    po = 64 * t
                    pt = pair_tiles[3] if t == 0 else pair_tiles.pop(3)
                    nc.sync.dma_start(out=stage_hs[t][:],
                                      in_=pt[po:po + 64, :])
                    ag(stage_hs[t], gath_hs[t])
                    for gi in range(2):
                        nc.gpsimd.dma_start(
                            out=outT_all[po:po + 64, gi * 4 + 3, :],
                            in_=gath_hs[t][gi, :, :],
                        )

                for h in range(NH):
                    outp = av_pool.tile([128, S], F32, tag="av")
                    if h < 4:
                        emit_av(h, outp)
                    else:
                        emit_scores_av(h, outp)
                    # evacuate: even head -> partitions 0..63, odd -> 64..127
                    if h % 2 == 0:
                        pt = pair_pool.tile([128, S], BF16, tag="pair")
                        pair_tiles[h // 2] = pt
                    else:
                        pt = pair_tiles[h // 2]
                    po = 64 * (h % 2)
                    nc.vector.tensor_copy(pt[po:po + 64, :], outp[0:64, :])
                    # raw sums row off PSUM as bf16 (broadcast matmul operand)
                    sums_t = sums_pool.tile([65, S], BF16, tag="sums")
                    sums_tiles[h] = sums_t
                    nc.vector.tensor_copy(sums_t[64:65, :], outp[64:65, :])
                    if debug and h == 0:
                        nc.sync.dma_start(out=dbg_sumsbf[64:65, :],
                                          in_=sums_t[64:65, :])
                    if h > 0:
                        normalize(h - 1)
                        if h - 1 in (1, 3, 5):
                            stage_pair((h - 1) // 2)
                        elif h - 1 == 6:
                            stage_head67(0)
                normalize(NH - 1)
                stage_head67(1)
                if debug:
                    nc.sync.dma_start(out=dbg_outall[:], in_=outT_all)

        # ---- output projection (contraction 128 over head pairs) ---------
        with tc.tile_pool(name="wo_ps", bufs=8, space="PSUM") as wo_ps_pool, \
             tc.tile_pool(name="osb", bufs=2) as o_pool:
            wops = [wo_ps_pool.tile([128, UH], F32, tag="wops",
                                    name=f"wops{mt}")
                    for mt in range(8)]
            # pairs 0..2 accumulate as their gathers land; the last pair is
            # applied mt-by-mt so evac/DMA of mt k overlaps matmuls of k+1
            for i, (p, gi) in enumerate(
                    [(p, gi) for p in range(3) for gi in range(2)]):
                for mt in range(8):
                    nc.tensor.matmul(
                        wops[mt],
                        outT_all[:, gi * 4 + p, mt * 128:(mt + 1) * 128],
                        wo_sb[:, gi * 4 + p, :],
                        start=(i == 0), stop=False,
                    )
            for mt in range(8):
                for gi in range(2):
                    nc.tensor.matmul(
                        wops[mt],
                        outT_all[:, gi * 4 + 3, mt * 128:(mt + 1) * 128],
                        wo_sb[:, gi * 4 + 3, :],
                        start=False, stop=(gi == 1),
                    )
                # bo is added on the host; ACT is idle here so it evacuates
                osb = o_pool.tile([128, UH], F32, tag="osb")
                nc.scalar.copy(osb, wops[mt])
                nc.sync.dma_start(
                    out=out_e[mt * 128:(mt + 1) * 128, :], in_=osb
                )

    nc.compile()
    return nc


def _get_nc():
    if "nc" not in _CACHED:
        _CACHED["nc"] = build_nc()
    return _CACHED["nc"]


def _swiz(a2d):
    """[dc*128, N] -> [128, dc, N] SBUF tile layout, contiguous."""
    n = a2d.shape[0] // 128
    return np.ascontiguousarray(
        a2d.reshape(n, 128, a2d.shape[1]).transpose(1, 0, 2))


def _in_maps(queries, keys, values, Wq, bq, Wk, bk, Wv, bv, Wo, bo, mk, mv):
    # x shipped transposed AND pre-swizzled: [128, dc, S]
    x_bf = [np.ascontiguousarray(a.transpose(0, 2, 1)).astype(NP_BF16)
            for a in (queries, keys, values)]
    w_bf = [np.ascontiguousarray(a).astype(NP_BF16) for a in (Wq, Wk, Wv)]
    wo_bf = np.ascontiguousarray(Wo).astype(NP_BF16)
    mkT_bf = np.ascontiguousarray(SCALE_M * mk.T).astype(NP_BF16)  # [HD, M]
    mv_bf = np.ascontiguousarray(SCALE_M * mv).astype(NP_BF16)
    maps = []
    for c in range(8):
        b, g = c // 2, c % 2
        sl = slice(g * HD, (g + 1) * HD)
        ul = slice(g * UH, (g + 1) * UH)
        # Wo packed by head pairs: row (G*8 + 2*hp + t)*64 + d ->
        # partition t*64+d, free (G*4+hp, c)
        wo_p = (wo_bf[:, ul].reshape(2, 4, 2, 64, UH)
                .transpose(2, 3, 0, 1, 4).reshape(128, 8, UH))
        maps.append({
            "xq": _swiz(x_bf[0][b]),
            "xk": _swiz(x_bf[1][b]),
            "xv": _swiz(x_bf[2][b]),
            "wq": _swiz(w_bf[0][:, sl]),
            "wk": _swiz(w_bf[1][:, sl]),
            "wv": _swiz(w_bf[2][:, sl]),
            "bq": np.ascontiguousarray(bq[sl].reshape(4, 128).T),
            "bk": np.ascontiguousarray(bk[sl].reshape(4, 128).T),
            "bv": np.ascontiguousarray(bv[sl]),
            "wo": np.ascontiguousarray(wo_p),
            "mk": _swiz(mkT_bf[sl, :]),
            "mv": np.ascontiguousarray(mv_bf[:, sl]),
        })
    return maps


def kernel(queries, keys, values, Wq, bq, Wk, bk, Wv, bv, Wo, bo, mk, mv, h=16,
           **_unused):
    queries = np.asarray(queries, np.float32)
    keys = np.asarray(keys, np.float32)
    values = np.asarray(values, np.float32)
    Wq = np.asarray(Wq, np.float32)
    Wk = np.asarray(Wk, np.float32)
    Wv = np.asarray(Wv, np.float32)
    Wo = np.asarray(Wo, np.float32)
    bq = np.asarray(bq, np.float32)
    bk = np.asarray(bk, np.float32)
    bv = np.asarray(bv, np.float32)
    bo = np.asarray(bo, np.float32)
    mk = np.asarray(mk, np.float32).reshape(M, -1)
    mv = np.asarray(mv, np.float32).reshape(M, -1)

    nc = _get_nc()
    in_maps = _in_maps(queries, keys, values, Wq, bq, Wk, bk, Wv, bv, Wo, bo,
                       mk, mv)

    trace = bool(int(os.environ.get("BASS_KERNEL_TRACE", "0")))
    res = run_bass_kernel_spmd(nc, in_maps, list(range(8)), trace=trace)
    _CACHED["last_result"] = res

    out = np.empty((B, S, UNITS), np.float32)
    for c in range(8):
        b, g = c // 2, c % 2
        out[b, :, g * UH:(g + 1) * UH] = res.results[c]["out"] + bo[g * UH:(g + 1) * UH]
    return out
